# revision 15
# baseline (speedup 1.0000x reference)
"""Multi-head self-attention (RoPE, causal) Trainium2 Bass kernel, 8 NeuronCores.

Sharding: data-parallel over batch (B=2) x tensor-parallel over heads
(16 heads -> 4 groups of 4). Core c handles batch b=c//4, heads 4*(c%4)..4*(c%4)+3.
Each core computes its 4 heads' attention plus a partial output projection;
the host sums the 4 partial outputs per batch element.

v2 layout (per core), engine-balance driven:
  x^T [1024d, L] bf16 (host pre-transposed). Q^T/K^T [256c, L] = W_slice @ x^T
  computed dt-OUTER over two L-halves (8 open PSUM accumulation groups), so
  the PE starts as soon as the first x d-tile lands and weight loads amortize.
  RoPE applied on bf16 SBUF tiles: scalar-engine PSUM->SBUF copy, then DVE
  shuffle/mul/mul/add (2x mode) -- the Pool engine is left free.
  Attention in transposed [k, q] layout per 512-wide q chunk, interleaved
  between the two projection halves so the activation engine (exp) overlaps
  the second half's projections. P^T = exp(T/8), no max subtraction.
  Causal: above-diagonal k-tiles skipped, diagonal ones masked (DVE).
  O'^T and softmax denominators (ones column folded into V) accumulate in
  PSUM over k tiles. Denominator reciprocal row is broadcast SBUF->SBUF.
  Output projection runs per q chunk right after its attention completes
  (no drain tail); output staging copies run on the idle Pool engine.
"""
import sys, math

sys.path.insert(0, "/opt/trn_rl_repo")

import numpy as np
import ml_dtypes

import concourse.bacc as bacc
import concourse.bass as bass
import concourse.mybir as mybir
import concourse.tile as tile
from concourse.bass_utils import run_bass_kernel_spmd

BF16 = mybir.dt.bfloat16
F32 = mybir.dt.float32
NPBF16 = ml_dtypes.bfloat16

D_MODEL = 1024
D_HEAD = 64
HALF = D_HEAD // 2
ROPE_THETA = 10000.0
N_CORES = 8
C = 256  # channels per core (4 heads x 64)
SWAP32 = [i ^ 1 for i in range(32)]


def _attention_chunk(nc, qc, qt_c, kt_c, vt_c, at, mks, atps, ops_, ptp, rip,
                     scrp, scale):
    """Causal attention for one 512-wide q chunk (both head pairs)."""
    qw = 512
    qs = qc * qw
    ktmax = (qs + qw) // 128
    for pair in range(2):
        po = ops_.tile([128, 1024], F32, tag="o", name=f"po_{pair}_{qc}")
        for kt in range(ktmax):
            off = kt * 128 - qs
            qlo = max(0, off)      # only q >= k contributes
            kc, ko = kt // 4, (kt % 4) * 128
            pt_ps = atps.tile([128, 1024], F32, tag="tps",
                              name=f"pt_{pair}_{qc}_{kt}")
            for hloc in range(2):
                nc.tensor.matmul(
                    pt_ps[:, 512 * hloc + qlo:512 * hloc + qw],
                    lhsT=kt_c[kc][64 * hloc:64 * hloc + 64, pair,
                                  ko:ko + 128],
                    rhs=qt_c[qc][64 * hloc:64 * hloc + 64, pair,
                                 qlo:qw],
                    start=True, stop=True,
                    tile_position=(64 * hloc, 0),
                    skip_group_check=True)
            pt_sb = ptp.tile([128, 1024], BF16, tag="p",
                             name=f"ptsb_{pair}_{qc}_{kt}")
            pv_ps = pt_ps[:, :].rearrange("p (h x) -> p h x", h=2)
            pv_sb = pt_sb[:, :].rearrange("p (h x) -> p h x", h=2)
            nc.scalar.activation(pv_sb[:, :, qlo:qw],
                                 pv_ps[:, :, qlo:qw],
                                 mybir.ActivationFunctionType.Exp,
                                 scale=scale)
            if off >= 0:
                # mask the 128-wide diagonal block (tril); rest kept
                for hloc in range(2):
                    nc.vector.tensor_mul(
                        pt_sb[:, 512 * hloc + qlo:512 * hloc + qlo + 128],
                        pt_sb[:, 512 * hloc + qlo:512 * hloc + qlo + 128],
                        mks[:, 0:128])
            for hloc in range(2):
                h = 2 * pair + hloc
                # lhsT [V_h | 1]: row 64 of the output accumulates the
                # softmax denominators for free
                nc.tensor.matmul(
                    po[0:65, 512 * hloc + qlo:512 * hloc + qw],
                    lhsT=vt_c[kc][:, kt % 4, 65 * h:65 * h + 65],
                    rhs=pt_sb[:, 512 * hloc + qlo:512 * hloc + qw],
                    start=(kt == 0), stop=(kt == ktmax - 1),
                    skip_group_check=True)
        # normalize: reciprocal of the denominator row (SBUF input -- the
        # custom-DVE recip can't read PSUM), broadcast via a DRAM bounce
        # (DMA partition-broadcast needs a DRAM source), scale
        rrow = rip.tile([1, 1024], F32, tag="rr", name=f"rr_{pair}_{qc}")
        nc.vector.tensor_copy(rrow[:], po[64:65, :])
        pbi = rip.tile([1, 1024], F32, tag="ri", name=f"pbi_{pair}_{qc}")
        nc.vector.reciprocal_approx_fast(out=pbi[:], in_=rrow[:])
        scrt = scrp.tile([1, 1024], F32, tag="scr", name=f"scr_{pair}_{qc}")
        nc.sync.dma_start(out=scrt[:], in_=pbi[:])
        pb = rip.tile([64, 1024], F32, tag="pb", name=f"pb_{pair}_{qc}")
        nc.sync.dma_start(out=pb[:], in_=scrt[:].partition_broadcast(64))
        tm = rip.tile([64, 1024], BF16, tag="tm", name=f"tm_{pair}_{qc}")
        nc.vector.tensor_mul(tm[:], po[0:64, :], pb[:])
        nc.vector.tensor_copy(at[0:64, pair, qs:qs + qw], tm[:, 0:qw])
        nc.sync.dma_start(out=at[64:128, pair, qs:qs + qw],
                          in_=tm[:, 512:512 + qw])


def _outproj_chunk(nc, qc, at, wo, out_d, ops_, osp):
    """Output projection + DMA for the 4 L-tiles of one q chunk."""
    for qtl in range(qc * 4, qc * 4 + 4):
        pout = ops_.tile([128, 1024], F32, tag="o", name=f"pout_{qtl}")
        for ct in range(2):
            for eh in range(2):
                nc.tensor.matmul(
                    pout[:, eh * 512:eh * 512 + 512],
                    lhsT=at[:, ct, qtl * 128:qtl * 128 + 128],
                    rhs=wo[:, ct, eh * 512:eh * 512 + 512],
                    start=(ct == 0), stop=(ct == 1),
                    skip_group_check=True)
        stg = osp.tile([128, 1024], F32, tag="stg", name=f"stg_{qtl}")
        nc.vector.tensor_copy(stg[:], pout[:])
        nc.sync.dma_start(out=out_d[qtl * 128:qtl * 128 + 128, :],
                          in_=stg[:])


def _body(nc, tc, L, pp, rtp, ptp, rip, osp):
    assert L == 2048
    scale = 1.0 / math.sqrt(D_HEAD)

    xt_d = nc.dram_tensor("xt", [D_MODEL, L], BF16, kind="ExternalInput").ap()
    wq_d = nc.dram_tensor("wqt", [D_MODEL, C], BF16, kind="ExternalInput").ap()
    wk_d = nc.dram_tensor("wkt", [D_MODEL, C], BF16, kind="ExternalInput").ap()
    wv_d = nc.dram_tensor("wvt", [D_MODEL, C], BF16, kind="ExternalInput").ap()
    wo_d = nc.dram_tensor("wot", [C, D_MODEL], BF16, kind="ExternalInput").ap()
    cos_d = nc.dram_tensor("cosb", [128, L], BF16, kind="ExternalInput").ap()
    sin_d = nc.dram_tensor("ssin", [128, L], BF16, kind="ExternalInput").ap()
    mk_d = nc.dram_tensor("masks", [128, 128], BF16,
                          kind="ExternalInput").ap()
    out_d = nc.dram_tensor("out", [L, D_MODEL], F32, kind="ExternalOutput").ap()

    # ---- persistent SBUF tensors
    wq = pp.tile([128, 8, C], BF16)
    wk = pp.tile([128, 8, C], BF16)
    wv = pp.tile([128, 8, C], BF16)
    wo = pp.tile([128, 2, D_MODEL], BF16)
    cs = pp.tile([128, L], BF16)
    sn = pp.tile([128, L], BF16)
    mks = pp.tile([128, 128], BF16)
    qt_c = [pp.tile([128, 2, 512], BF16, name=f"qt{i}") for i in range(4)]
    kt_c = [pp.tile([128, 2, 512], BF16, name=f"ktc{i}") for i in range(4)]
    vt_c = [pp.tile([128, 4, C + 4], BF16, name=f"vt{i}") for i in range(4)]
    at = pp.tile([128, 2, L], BF16)
    xts = [pp.tile([128, L], BF16, name=f"xt{i}") for i in range(8)]

    # ---- loads (weights for the first matmuls lead; x d-tiles streamed)
    nc.sync.dma_start(out=wq[:], in_=wq_d.rearrange("(a p) c -> p a c", p=128))
    nc.sync.dma_start(out=wk[:], in_=wk_d.rearrange("(a p) c -> p a c", p=128))
    for i in range(8):
        nc.sync.dma_start(out=xts[i][:], in_=xt_d[i * 128:(i + 1) * 128, :])
    nc.sync.dma_start(out=cs[:], in_=cos_d)
    nc.sync.dma_start(out=sn[:], in_=sin_d)
    nc.sync.dma_start(out=wv[:], in_=wv_d.rearrange("(a p) c -> p a c", p=128))
    nc.sync.dma_start(out=mks[:], in_=mk_d)
    nc.sync.dma_start(out=wo[:], in_=wo_d.rearrange("(a p) e -> p a e", p=128))
    for i in range(4):
        ov = vt_c[i][:, :, :].rearrange("p l (h x) -> p l h x", x=65)
        nc.gpsimd.memset(ov[:, :, :, 64], 1.0)

    for H in range(2):
        # ---- Q^T / K^T projection for L-half H, dt-outer: 8 open PSUM
        # accumulation groups; each weight tile feeds 2 q chunks.
        with tc.tile_pool(name=f"qk_ps{H}", bufs=8, space="PSUM") as qkps:
            # RoPE/drain order: K chunks first (both needed by the first
            # attention chunk), then Q of the later chunk, then Q of the
            # earlier one -- attention runs chunks (2H+1, 2H+0).
            order = [("k", 0), ("k", 1), ("q", 1), ("q", 0)]
            ps = {}
            for nm, i in order:
                for ct in (0, 1):
                    ps[(nm, ct, i)] = qkps.tile(
                        [128, 512], F32, tag="qkps",
                        name=f"ps_{nm}{ct}_{2 * H + i}")
            wmap = {"q": wq, "k": wk}
            for dt_ in range(7):
                for nm, i in order:
                    for ct in (0, 1):
                        qc = 2 * H + i
                        nc.tensor.matmul(
                            ps[(nm, ct, i)][:],
                            lhsT=wmap[nm][:, dt_, ct * 128:ct * 128 + 128],
                            rhs=xts[dt_][:, qc * 512:qc * 512 + 512],
                            start=(dt_ == 0), stop=False,
                            skip_group_check=True)
            # staggered last accumulation step: finish each group and
            # immediately drain it (PSUM -> bf16, copies split across the
            # scalar and vector engines) so PSUM banks free progressively
            # and the RoPE chain starts while later groups still matmul.
            for gi, (nm, i) in enumerate(order):
                for ct in (0, 1):
                    qc = 2 * H + i
                    ls = qc * 512
                    p = ps[(nm, ct, i)]
                    nc.tensor.matmul(
                        p[:],
                        lhsT=wmap[nm][:, 7, ct * 128:ct * 128 + 128],
                        rhs=xts[7][:, qc * 512:qc * 512 + 512],
                        start=False, stop=True,
                        skip_group_check=True)
                    cb = rtp.tile([128, 512], BF16, tag="t",
                                  name=f"cb_{nm}{ct}{qc}")
                    if ct == 0:
                        nc.scalar.copy(cb[:], p[:])
                    else:
                        nc.vector.tensor_copy(cb[:], p[:])
                    sh = rtp.tile([128, 512], BF16, tag="t",
                                  name=f"sh_{nm}{ct}{qc}")
                    nc.vector.stream_shuffle(sh[:], cb[:], SWAP32)
                    t1 = rtp.tile([128, 512], BF16, tag="t",
                                  name=f"t1_{nm}{ct}{qc}")
                    nc.vector.tensor_mul(t1[:], cb[:], cs[:, ls:ls + 512])
                    t2 = rtp.tile([128, 512], BF16, tag="t",
                                  name=f"t2_{nm}{ct}{qc}")
                    nc.vector.tensor_mul(t2[:], sh[:], sn[:, ls:ls + 512])
                    dstc = qt_c if nm == "q" else kt_c
                    nc.vector.tensor_add(dstc[qc][:, ct, :], t1[:], t2[:])
        # ---- V projection for this half's 8 L-tiles (x stationary)
        with tc.tile_pool(name=f"v_ps{H}", bufs=2, space="PSUM") as vps:
            for lt in range(8 * H, 8 * H + 8):
                pv = vps.tile([128, C], F32, tag="vps", name=f"pv_{lt}")
                for dt_ in range(8):
                    nc.tensor.matmul(
                        pv[:],
                        lhsT=xts[dt_][:, lt * 128:lt * 128 + 128],
                        rhs=wv[:, dt_, :],
                        start=(dt_ == 0), stop=(dt_ == 7))
                ov = vt_c[lt // 4][:, lt % 4, :].rearrange(
                    "p (h x) -> p h x", x=65)[:, :, 0:64]
                nc.scalar.copy(ov, pv[:].rearrange("p (h x) -> p h x", x=64))
        # ---- attention + output projection for this half's two q chunks
        with tc.tile_pool(name=f"att_ps{H}", bufs=2, space="PSUM") as atps, \
             tc.tile_pool(name=f"o_ps{H}", bufs=2, space="PSUM") as ops_, \
             tc.tile_pool(name=f"riscr{H}", bufs=4, space="DRAM") as scrp:
            for i in (1, 0):  # bigger chunk first: its drain overlaps the next
                qc = 2 * H + i
                _attention_chunk(nc, qc, qt_c, kt_c, vt_c, at, mks, atps,
                                 ops_, ptp, rip, scrp, scale)
                _outproj_chunk(nc, qc, at, wo, out_d, ops_, osp)


def build_nc(L=2048):
    """Build + compile the per-core Bass program (same NEFF on all 8 cores)."""
    assert L % 256 == 0
    nc = bacc.Bacc("TRN2", target_bir_lowering=False, debug=False,
                   num_devices=N_CORES)
    with tile.TileContext(nc) as tc:
        with tc.tile_pool(name="persist", bufs=1) as pp, \
             tc.tile_pool(name="ropet", bufs=8) as rtp, \
             tc.tile_pool(name="ptp", bufs=3) as ptp, \
             tc.tile_pool(name="rinvp", bufs=2) as rip, \
             tc.tile_pool(name="ostg", bufs=3) as osp:
            _body(nc, tc, L, pp, rtp, ptp, rip, osp)
    nc.compile()
    return nc


_NC_CACHE = {}


def _get_nc(L):
    if L not in _NC_CACHE:
        _NC_CACHE[L] = build_nc(L)
    return _NC_CACHE[L]


def make_inputs(x, token_positions, Wq, Wk, Wv, Wo):
    """Host-side shard/layout prep -> list of 8 per-core input dicts."""
    B, L, _ = x.shape
    pos = np.asarray(token_positions).astype(np.float64)
    S = ROPE_THETA ** (-2.0 / D_HEAD)
    thetas = S ** np.arange(HALF, dtype=np.float64)
    ang = pos[:, None] * thetas[None, :]          # [L, 32]
    cosL = np.cos(ang).T                          # [32, L]
    sinL = np.sin(ang).T
    # per-channel tables on the natural (head, dim) layout:
    # row p (within a 64-row head block): pair i = (p%64)//2
    # cosb[p] = cos(theta_i * pos); ssin[p] = -sin if dim even else +sin
    cosb = np.empty((128, L), dtype=np.float64)
    ssin = np.empty((128, L), dtype=np.float64)
    for p in range(128):
        i = (p % 64) // 2
        cosb[p] = cosL[i]
        ssin[p] = -sinL[i] if (p % 2 == 0) else sinL[i]
    cosb = cosb.astype(NPBF16)
    ssin = ssin.astype(NPBF16)

    r = np.arange(128)[:, None]
    col = np.arange(128)[None, :]
    masks = (col >= r).astype(NPBF16)  # [128, 128] tril(keep q>=k)

    xts = [np.ascontiguousarray(x[b].astype(NPBF16).T) for b in range(B)]
    in_maps = []
    shard_cache = {}
    for core in range(N_CORES):
        b, hg = core // 4, core % 4
        if hg not in shard_cache:
            rows = slice(hg * 256, hg * 256 + 256)
            shard_cache[hg] = {
                "wqt": np.ascontiguousarray(Wq[rows].astype(NPBF16).T),
                "wkt": np.ascontiguousarray(Wk[rows].astype(NPBF16).T),
                "wvt": np.ascontiguousarray(Wv[rows].astype(NPBF16).T),
                "wot": np.ascontiguousarray(Wo[:, rows].astype(NPBF16).T),
            }
        m = dict(shard_cache[hg])
        m["xt"] = xts[b]
        m["cosb"] = cosb
        m["ssin"] = ssin
        m["masks"] = masks
        in_maps.append(m)
    return in_maps


def kernel(x, token_positions, Wq, Wk, Wv, Wo):
    x = np.asarray(x); Wq = np.asarray(Wq); Wk = np.asarray(Wk)
    Wv = np.asarray(Wv); Wo = np.asarray(Wo)
    B, L, _ = x.shape
    nc = _get_nc(L)
    in_maps = make_inputs(x, token_positions, Wq, Wk, Wv, Wo)
    res = run_bass_kernel_spmd(nc, in_maps, core_ids=list(range(N_CORES)))
    out = np.zeros((B, L, D_MODEL), dtype=np.float32)
    for core in range(N_CORES):
        out[core // 4] += res.results[core]["out"]
    return out


# revision 18
# speedup vs baseline: 1.2140x; 1.2140x over previous
"""Multi-head self-attention (RoPE, causal) Trainium2 Bass kernel, 8 NeuronCores.

Sharding: data-parallel over batch (B=2) x tensor-parallel over heads
(16 heads -> 4 groups of 4). Core c handles batch b=c//4, heads 4*(c%4)..4*(c%4)+3.
Each core computes its 4 heads' attention plus a partial output projection;
the host sums the 4 partial outputs per batch element.

v2 layout (per core), engine-balance driven:
  x^T [1024d, L] bf16 (host pre-transposed). Q^T/K^T [256c, L] = W_slice @ x^T
  computed dt-OUTER over two L-halves (8 open PSUM accumulation groups), so
  the PE starts as soon as the first x d-tile lands and weight loads amortize.
  RoPE applied on bf16 SBUF tiles: scalar-engine PSUM->SBUF copy, then DVE
  shuffle/mul/mul/add (2x mode) -- the Pool engine is left free.
  Attention in transposed [k, q] layout per 512-wide q chunk, interleaved
  between the two projection halves so the activation engine (exp) overlaps
  the second half's projections. P^T = exp(T/8), no max subtraction.
  Causal: above-diagonal k-tiles skipped, diagonal ones masked (DVE).
  O'^T and softmax denominators (ones column folded into V) accumulate in
  PSUM over k tiles. Denominator reciprocal row is broadcast SBUF->SBUF.
  Output projection runs per q chunk right after its attention completes
  (no drain tail); output staging copies run on the idle Pool engine.
"""
import sys, math

sys.path.insert(0, "/opt/trn_rl_repo")

import numpy as np
import ml_dtypes

import concourse.bacc as bacc
import concourse.bass as bass
import concourse.mybir as mybir
import concourse.tile as tile
from concourse.bass_utils import run_bass_kernel_spmd

BF16 = mybir.dt.bfloat16
F32 = mybir.dt.float32
NPBF16 = ml_dtypes.bfloat16

D_MODEL = 1024
D_HEAD = 64
HALF = D_HEAD // 2
ROPE_THETA = 10000.0
N_CORES = 8
C = 256  # channels per core (4 heads x 64)
SWAP32 = [i ^ 1 for i in range(32)]


def _attention_chunk(nc, qc, qt_c, kt_c, vt_c, at, mks, atps, ops_, ptp, rip,
                     scrp, scale):
    """Causal attention for one 512-wide q chunk (both head pairs)."""
    qw = 512
    qs = qc * qw
    ktmax = (qs + qw) // 128
    for pair in range(2):
        po = ops_.tile([128, 1024], F32, tag="o", name=f"po_{pair}_{qc}")
        for kt in range(ktmax):
            off = kt * 128 - qs
            qlo = max(0, off)      # only q >= k contributes
            kc, ko = kt // 4, (kt % 4) * 128
            pt_ps = atps.tile([128, 1024], F32, tag="tps",
                              name=f"pt_{pair}_{qc}_{kt}")
            for hloc in range(2):
                nc.tensor.matmul(
                    pt_ps[:, 512 * hloc + qlo:512 * hloc + qw],
                    lhsT=kt_c[kc][64 * hloc:64 * hloc + 64, pair,
                                  ko:ko + 128],
                    rhs=qt_c[qc][64 * hloc:64 * hloc + 64, pair,
                                 qlo:qw],
                    start=True, stop=True,
                    tile_position=(64 * hloc, 0),
                    skip_group_check=True)
            pt_sb = ptp.tile([128, 1024], BF16, tag="p",
                             name=f"ptsb_{pair}_{qc}_{kt}")
            pv_ps = pt_ps[:, :].rearrange("p (h x) -> p h x", h=2)
            pv_sb = pt_sb[:, :].rearrange("p (h x) -> p h x", h=2)
            nc.scalar.activation(pv_sb[:, :, qlo:qw],
                                 pv_ps[:, :, qlo:qw],
                                 mybir.ActivationFunctionType.Exp,
                                 scale=scale)
            if off >= 0:
                # mask the 128-wide diagonal block (tril) on the idle Pool
                # engine; the DVE queue carries the normalize chains
                for hloc in range(2):
                    nc.gpsimd.tensor_mul(
                        pt_sb[:, 512 * hloc + qlo:512 * hloc + qlo + 128],
                        pt_sb[:, 512 * hloc + qlo:512 * hloc + qlo + 128],
                        mks[:, 0:128])
            for hloc in range(2):
                h = 2 * pair + hloc
                # lhsT [V_h | 1]: row 64 of the output accumulates the
                # softmax denominators for free
                nc.tensor.matmul(
                    po[0:65, 512 * hloc + qlo:512 * hloc + qw],
                    lhsT=vt_c[kc][:, kt % 4, 65 * h:65 * h + 65],
                    rhs=pt_sb[:, 512 * hloc + qlo:512 * hloc + qw],
                    start=(kt == 0), stop=(kt == ktmax - 1),
                    skip_group_check=True)
        # normalize: reciprocal of the denominator row (SBUF input -- the
        # custom-DVE recip can't read PSUM), broadcast via a DRAM bounce
        # (DMA partition-broadcast needs a DRAM source), scale
        rrow = rip.tile([1, 1024], F32, tag="rr", name=f"rr_{pair}_{qc}")
        nc.vector.tensor_copy(rrow[:], po[64:65, :])
        pbi = rip.tile([1, 1024], F32, tag="ri", name=f"pbi_{pair}_{qc}")
        nc.vector.reciprocal_approx_fast(out=pbi[:], in_=rrow[:])
        scrt = scrp.tile([1, 1024], F32, tag="scr", name=f"scr_{pair}_{qc}")
        nc.sync.dma_start(out=scrt[:], in_=pbi[:])
        pb = rip.tile([64, 1024], F32, tag="pb", name=f"pb_{pair}_{qc}")
        nc.sync.dma_start(out=pb[:], in_=scrt[:].partition_broadcast(64))
        tm = rip.tile([64, 1024], BF16, tag="tm", name=f"tm_{pair}_{qc}")
        nc.vector.tensor_mul(tm[:], po[0:64, :], pb[:])
        nc.vector.tensor_copy(at[0:64, pair, qs:qs + qw], tm[:, 0:qw])
        nc.sync.dma_start(out=at[64:128, pair, qs:qs + qw],
                          in_=tm[:, 512:512 + qw])


def _outproj_chunk(nc, qc, at, wo, out_d, ops_, osp):
    """Output projection + DMA for the 4 L-tiles of one q chunk."""
    for qtl in range(qc * 4, qc * 4 + 4):
        pout = ops_.tile([128, 1024], F32, tag="o", name=f"pout_{qtl}")
        for ct in range(2):
            for eh in range(2):
                nc.tensor.matmul(
                    pout[:, eh * 512:eh * 512 + 512],
                    lhsT=at[:, ct, qtl * 128:qtl * 128 + 128],
                    rhs=wo[:, ct, eh * 512:eh * 512 + 512],
                    start=(ct == 0), stop=(ct == 1),
                    skip_group_check=True)
        stg = osp.tile([128, 1024], F32, tag="stg", name=f"stg_{qtl}")
        nc.vector.tensor_copy(stg[:], pout[:])
        nc.gpsimd.dma_start(out=out_d[qtl * 128:qtl * 128 + 128, :],
                            in_=stg[:])


def _body(nc, tc, L, pp, rtp, ptp, rip, osp):
    assert L == 2048
    scale = 1.0 / math.sqrt(D_HEAD)

    xt_d = nc.dram_tensor("xt", [D_MODEL, L], BF16, kind="ExternalInput").ap()
    wq_d = nc.dram_tensor("wqt", [D_MODEL, C], BF16, kind="ExternalInput").ap()
    wk_d = nc.dram_tensor("wkt", [D_MODEL, C], BF16, kind="ExternalInput").ap()
    wv_d = nc.dram_tensor("wvt", [D_MODEL, C], BF16, kind="ExternalInput").ap()
    wo_d = nc.dram_tensor("wot", [C, D_MODEL], BF16, kind="ExternalInput").ap()
    cos_d = nc.dram_tensor("cosb", [128, L], BF16, kind="ExternalInput").ap()
    sin_d = nc.dram_tensor("ssin", [128, L], BF16, kind="ExternalInput").ap()
    mk_d = nc.dram_tensor("masks", [128, 128], BF16,
                          kind="ExternalInput").ap()
    out_d = nc.dram_tensor("out", [L, D_MODEL], F32, kind="ExternalOutput").ap()

    # ---- persistent SBUF tensors
    wq = pp.tile([128, 8, C], BF16)
    wk = pp.tile([128, 8, C], BF16)
    wv = pp.tile([128, 8, C], BF16)
    wo = pp.tile([128, 2, D_MODEL], BF16)
    cs = pp.tile([128, L], BF16)
    sn = pp.tile([128, L], BF16)
    mks = pp.tile([128, 128], BF16)
    qt_c = [pp.tile([128, 2, 512], BF16, name=f"qt{i}") for i in range(4)]
    kt_c = [pp.tile([128, 2, 512], BF16, name=f"ktc{i}") for i in range(4)]
    vt_c = [pp.tile([128, 4, C + 4], BF16, name=f"vt{i}") for i in range(4)]
    at = pp.tile([128, 2, L], BF16)
    xts = [pp.tile([128, L], BF16, name=f"xt{i}") for i in range(8)]

    # ---- loads (weights for the first matmuls lead; x d-tiles streamed)
    nc.sync.dma_start(out=wq[:], in_=wq_d.rearrange("(a p) c -> p a c", p=128))
    nc.sync.dma_start(out=wk[:], in_=wk_d.rearrange("(a p) c -> p a c", p=128))
    for i in range(8):
        nc.sync.dma_start(out=xts[i][:], in_=xt_d[i * 128:(i + 1) * 128, :])
    nc.sync.dma_start(out=cs[:], in_=cos_d)
    nc.sync.dma_start(out=sn[:], in_=sin_d)
    nc.sync.dma_start(out=wv[:], in_=wv_d.rearrange("(a p) c -> p a c", p=128))
    nc.sync.dma_start(out=mks[:], in_=mk_d)
    nc.sync.dma_start(out=wo[:], in_=wo_d.rearrange("(a p) e -> p a e", p=128))
    for i in range(4):
        ov = vt_c[i][:, :, :].rearrange("p l (h x) -> p l h x", x=65)
        nc.gpsimd.memset(ov[:, :, :, 64], 1.0)

    for H in range(2):
        # ---- Q^T / K^T projection for L-half H, dt-outer: 8 open PSUM
        # accumulation groups; each weight tile feeds 2 q chunks.
        with tc.tile_pool(name=f"qk_ps{H}", bufs=8, space="PSUM") as qkps:
            # RoPE/drain order: K chunks first (both needed by the first
            # attention chunk), then Q of the later chunk, then Q of the
            # earlier one -- attention runs chunks (2H+1, 2H+0).
            order = [("k", 0), ("k", 1), ("q", 1), ("q", 0)]
            ps = {}
            for nm, i in order:
                for ct in (0, 1):
                    ps[(nm, ct, i)] = qkps.tile(
                        [128, 512], F32, tag="qkps",
                        name=f"ps_{nm}{ct}_{2 * H + i}")
            wmap = {"q": wq, "k": wk}
            for dt_ in range(7):
                for nm, i in order:
                    for ct in (0, 1):
                        qc = 2 * H + i
                        nc.tensor.matmul(
                            ps[(nm, ct, i)][:],
                            lhsT=wmap[nm][:, dt_, ct * 128:ct * 128 + 128],
                            rhs=xts[dt_][:, qc * 512:qc * 512 + 512],
                            start=(dt_ == 0), stop=False,
                            skip_group_check=True)
            # staggered last accumulation step: finish each group and
            # immediately drain it (PSUM -> bf16 on the scalar engine, whose
            # queue has no other work here) so PSUM banks free progressively
            # -- the V matmuls reuse them while the DVE chews the RoPE math.
            cbs = {}
            for nm, i in order:
                for ct in (0, 1):
                    qc = 2 * H + i
                    p = ps[(nm, ct, i)]
                    nc.tensor.matmul(
                        p[:],
                        lhsT=wmap[nm][:, 7, ct * 128:ct * 128 + 128],
                        rhs=xts[7][:, qc * 512:qc * 512 + 512],
                        start=False, stop=True,
                        skip_group_check=True)
                    cb = rtp.tile([128, 512], BF16, tag="t",
                                  name=f"cb_{nm}{ct}{qc}")
                    nc.scalar.copy(cb[:], p[:])
                    cbs[(nm, ct, i)] = cb
            for nm, i in order:
                for ct in (0, 1):
                    qc = 2 * H + i
                    ls = qc * 512
                    cb = cbs[(nm, ct, i)]
                    sh = rtp.tile([128, 512], BF16, tag="s",
                                  name=f"sh_{nm}{ct}{qc}")
                    nc.vector.stream_shuffle(sh[:], cb[:], SWAP32)
                    t1 = rtp.tile([128, 512], BF16, tag="s",
                                  name=f"t1_{nm}{ct}{qc}")
                    nc.vector.tensor_mul(t1[:], cb[:], cs[:, ls:ls + 512])
                    t2 = rtp.tile([128, 512], BF16, tag="s",
                                  name=f"t2_{nm}{ct}{qc}")
                    nc.vector.tensor_mul(t2[:], sh[:], sn[:, ls:ls + 512])
                    dstc = qt_c if nm == "q" else kt_c
                    nc.vector.tensor_add(dstc[qc][:, ct, :], t1[:], t2[:])
        # ---- V projection for this half's 8 L-tiles (x stationary)
        with tc.tile_pool(name=f"v_ps{H}", bufs=2, space="PSUM") as vps:
            for lt in range(8 * H, 8 * H + 8):
                pv = vps.tile([128, C], F32, tag="vps", name=f"pv_{lt}")
                for dt_ in range(8):
                    nc.tensor.matmul(
                        pv[:],
                        lhsT=xts[dt_][:, lt * 128:lt * 128 + 128],
                        rhs=wv[:, dt_, :],
                        start=(dt_ == 0), stop=(dt_ == 7))
                ov = vt_c[lt // 4][:, lt % 4, :].rearrange(
                    "p (h x) -> p h x", x=65)[:, :, 0:64]
                nc.scalar.copy(ov, pv[:].rearrange("p (h x) -> p h x", x=64))
        # ---- attention + output projection for this half's two q chunks
        with tc.tile_pool(name=f"att_ps{H}", bufs=2, space="PSUM") as atps, \
             tc.tile_pool(name=f"o_ps{H}", bufs=2, space="PSUM") as ops_, \
             tc.tile_pool(name=f"riscr{H}", bufs=4, space="DRAM") as scrp:
            for i in (1, 0):  # bigger chunk first: its drain overlaps the next
                qc = 2 * H + i
                _attention_chunk(nc, qc, qt_c, kt_c, vt_c, at, mks, atps,
                                 ops_, ptp, rip, scrp, scale)
                _outproj_chunk(nc, qc, at, wo, out_d, ops_, osp)


def build_nc(L=2048):
    """Build + compile the per-core Bass program (same NEFF on all 8 cores)."""
    assert L % 256 == 0
    nc = bacc.Bacc("TRN2", target_bir_lowering=False, debug=False,
                   num_devices=N_CORES)
    with tile.TileContext(nc) as tc:
        with tc.tile_pool(name="persist", bufs=1) as pp, \
             tc.tile_pool(name="ropet", bufs=8) as rtp, \
             tc.tile_pool(name="ptp", bufs=3) as ptp, \
             tc.tile_pool(name="rinvp", bufs=2) as rip, \
             tc.tile_pool(name="ostg", bufs=3) as osp:
            _body(nc, tc, L, pp, rtp, ptp, rip, osp)
    nc.compile()
    return nc


_NC_CACHE = {}


def _get_nc(L):
    if L not in _NC_CACHE:
        _NC_CACHE[L] = build_nc(L)
    return _NC_CACHE[L]


def make_inputs(x, token_positions, Wq, Wk, Wv, Wo):
    """Host-side shard/layout prep -> list of 8 per-core input dicts."""
    B, L, _ = x.shape
    pos = np.asarray(token_positions).astype(np.float64)
    S = ROPE_THETA ** (-2.0 / D_HEAD)
    thetas = S ** np.arange(HALF, dtype=np.float64)
    ang = pos[:, None] * thetas[None, :]          # [L, 32]
    cosL = np.cos(ang).T                          # [32, L]
    sinL = np.sin(ang).T
    # per-channel tables on the natural (head, dim) layout:
    # row p (within a 64-row head block): pair i = (p%64)//2
    # cosb[p] = cos(theta_i * pos); ssin[p] = -sin if dim even else +sin
    cosb = np.empty((128, L), dtype=np.float64)
    ssin = np.empty((128, L), dtype=np.float64)
    for p in range(128):
        i = (p % 64) // 2
        cosb[p] = cosL[i]
        ssin[p] = -sinL[i] if (p % 2 == 0) else sinL[i]
    cosb = cosb.astype(NPBF16)
    ssin = ssin.astype(NPBF16)

    r = np.arange(128)[:, None]
    col = np.arange(128)[None, :]
    masks = (col >= r).astype(NPBF16)  # [128, 128] tril(keep q>=k)

    xts = [np.ascontiguousarray(x[b].astype(NPBF16).T) for b in range(B)]
    in_maps = []
    shard_cache = {}
    for core in range(N_CORES):
        b, hg = core // 4, core % 4
        if hg not in shard_cache:
            rows = slice(hg * 256, hg * 256 + 256)
            shard_cache[hg] = {
                "wqt": np.ascontiguousarray(Wq[rows].astype(NPBF16).T),
                "wkt": np.ascontiguousarray(Wk[rows].astype(NPBF16).T),
                "wvt": np.ascontiguousarray(Wv[rows].astype(NPBF16).T),
                "wot": np.ascontiguousarray(Wo[:, rows].astype(NPBF16).T),
            }
        m = dict(shard_cache[hg])
        m["xt"] = xts[b]
        m["cosb"] = cosb
        m["ssin"] = ssin
        m["masks"] = masks
        in_maps.append(m)
    return in_maps


def kernel(x, token_positions, Wq, Wk, Wv, Wo):
    x = np.asarray(x); Wq = np.asarray(Wq); Wk = np.asarray(Wk)
    Wv = np.asarray(Wv); Wo = np.asarray(Wo)
    B, L, _ = x.shape
    nc = _get_nc(L)
    in_maps = make_inputs(x, token_positions, Wq, Wk, Wv, Wo)
    res = run_bass_kernel_spmd(nc, in_maps, core_ids=list(range(N_CORES)))
    out = np.zeros((B, L, D_MODEL), dtype=np.float32)
    for core in range(N_CORES):
        out[core // 4] += res.results[core]["out"]
    return out
